# revision 14
# baseline (speedup 1.0000x reference)
"""Trainium2 Bass kernel: full (non-causal) softmax attention.

Input:  query/key/value [1, 4096, 16, 128] f32 (B, S, H, D).
Output: [1, 4096, 16, 128] f32 = softmax(Q K^T / sqrt(D)) V per head.

Sharding: 16 heads over 8 cores -> 2 heads per core, no collectives.
Host pre-transposes Q,K per head to [D, S] in fp16; the device returns
the UN-normalized attention output [D, 512] per (head, q-chunk) plus
fp16 partial denominator accumulators [128, 4, 512]; the host reduces
the accumulators (sum over 128 k-lanes x 4 slots) and does the final
divide (cheap numpy).

Device pipeline (ACT-exp is the throughput floor, ~246us/core):
  - global stream of 512 score chunks (2 heads x 8 q-chunks x 32 kt),
    grouped into alternating 4-bank / 3-bank PSUM super-tiles
  - per group: PE writes scores (fp16 matmuls, N=512/bank), one big
    ACT exp (N=2048/1536, fp32 psum -> fp16 sbuf), then PE PV matmuls
    accumulate into a single out bank; DVE accumulates the softmax
    denominator with fp16 2x-mode adds into per-job accumulators.
  - software-pipelined by one group so PE never waits on ACT.
"""

import sys
from contextlib import ExitStack

import numpy as np

sys.path.insert(0, "/opt/trn_rl_repo")

import concourse.bacc as bacc
import concourse.bass as bass
import concourse.tile as tile
from concourse import mybir
from concourse.bass_utils import run_bass_kernel_spmd

N_CORES = 8
S = 4096
H = 16
D = 128
HPC = H // N_CORES   # heads per core = 2
QC = 512             # queries per job (one psum bank of fp32)
NQC = S // QC        # 8 q-chunks per head
KT = 128             # keys per score chunk (psum partition dim)
NKT = S // KT        # 32 key chunks
SCALE = float(D) ** -0.5
GA = 4               # chunks per group in buffer A (4 psum banks)
GB = 3               # chunks per group in buffer B (3 psum banks)

F32 = mybir.dt.float32
F16 = mybir.dt.float16


def _make_groups():
    """Global chunk stream split into alternating A/B groups.

    The remainder (size-1) group leads the stream: a 1-chunk group gets
    the first exp onto ACT as soon as its table load finishes.
    """
    chunks = [(h, qc, kt)
              for h in range(HPC) for qc in range(NQC) for kt in range(NKT)]
    # [1, 2, 4, 3, 4, 3, ...]: the leading 1 starts ACT early; the
    # shrunken second group keeps job boundaries off group boundaries
    # (avoids a PE hiccup on the shared out/acc resources every job).
    sizes = [1, 2]
    use_a = True
    left = len(chunks) - 3
    while left:
        n = min(GA if use_a else GB, left)
        sizes.append(n)
        left -= n
        use_a = not use_a
    groups = []
    i = 0
    for gi, n in enumerate(sizes):
        groups.append((gi % 2 == 0, chunks[i:i + n]))
        i += n
    return groups


def build_program():
    nc = bacc.Bacc("TRN2", target_bir_lowering=False, debug=False,
                   num_devices=N_CORES)

    qt_d = nc.dram_tensor("qt", [HPC, D, S], F16, kind="ExternalInput")
    kt_d = nc.dram_tensor("kt", [HPC, D, S], F16, kind="ExternalInput")
    v_d = nc.dram_tensor("v", [HPC, 128, NKT, D], F16, kind="ExternalInput")
    out_d = nc.dram_tensor("out", [HPC, NQC, D, QC], F16,
                           kind="ExternalOutput")
    acc_d = nc.dram_tensor("acc", [HPC, NQC, 128, 2, QC], F16,
                           kind="ExternalOutput")

    groups = _make_groups()

    with tile.TileContext(nc) as tc, ExitStack() as ctx:
        consts = ctx.enter_context(tc.tile_pool(name="consts", bufs=1))
        qkv_pool = ctx.enter_context(tc.tile_pool(name="qkv", bufs=2))
        pt_pool = ctx.enter_context(tc.tile_pool(name="pt", bufs=5))
        acc_pool = ctx.enter_context(tc.tile_pool(name="acc", bufs=2))
        osb_pool = ctx.enter_context(tc.tile_pool(name="osb", bufs=3))
        stA_pool = ctx.enter_context(
            tc.tile_pool(name="stA", bufs=1, space="PSUM"))
        stB_pool = ctx.enter_context(
            tc.tile_pool(name="stB", bufs=1, space="PSUM"))
        outp_pool = ctx.enter_context(
            tc.tile_pool(name="outp", bufs=1, space="PSUM"))

        # ---- warmup: ACT table load + PE HAM ramp while DMAs run ----
        wsrc = consts.tile([128, 16], F32, tag="wsrc")
        nc.vector.memset(wsrc[:], 0.0)
        wdst = consts.tile([128, 16], F16, tag="wdst")
        nc.scalar.activation(wdst[:], wsrc[:],
                             mybir.ActivationFunctionType.Exp, scale=SCALE)


        # ---- per-head staging ----
        def load_head(h, chunk_first=False):
            qt_sb = qkv_pool.tile([D, S], F16, tag="qt")
            kt_sb = qkv_pool.tile([D, S], F16, tag="kt")
            v_sb = qkv_pool.tile([128, NKT, D], F16, tag="v")
            if chunk_first:
                # staged so operands land in consumption order: kt/qt
                # prefix (first scores), v prefix (first PV), kt bulk
                # (first job walks all of kt), v bulk, qt remainder
                # (only needed at job 1, ~18us in)
                nc.sync.dma_start(out=kt_sb[:, 0:128], in_=kt_d[h][:, 0:128])
                nc.sync.dma_start(out=qt_sb[:, 0:QC], in_=qt_d[h][:, 0:QC])
                nc.sync.dma_start(out=kt_sb[:, 128:1024],
                                  in_=kt_d[h][:, 128:1024])
                nc.sync.dma_start(out=v_sb[:, 0:8, :], in_=v_d[h][:, 0:8, :])
                nc.sync.dma_start(out=kt_sb[:, 1024:], in_=kt_d[h][:, 1024:])
                nc.sync.dma_start(out=v_sb[:, 8:, :], in_=v_d[h][:, 8:, :])
                nc.sync.dma_start(out=qt_sb[:, QC:], in_=qt_d[h][:, QC:])
            else:
                nc.sync.dma_start(out=qt_sb[:], in_=qt_d[h])
                nc.sync.dma_start(out=kt_sb[:], in_=kt_d[h])
                nc.sync.dma_start(out=v_sb[:], in_=v_d[h])
            return qt_sb, kt_sb, v_sb

        heads_sb = [load_head(0, chunk_first=True), None]
        jobs = {}  # (h, qc) -> dict(out_ps, acc)

        def emit_pv_den(group_chunks, pt):
            # split the group by (h, qc) job; emit PV matmuls + den adds
            subs = []
            for i, (h, qc, kt) in enumerate(group_chunks):
                if subs and subs[-1][0] == (h, qc):
                    subs[-1][1].append((i, kt))
                else:
                    subs.append(((h, qc), [(i, kt)]))
            for (h, qc), items in subs:
                i0 = items[0][0]
                n = len(items)
                job = jobs.get((h, qc))
                fresh = job is None
                if fresh:
                    out_ps = outp_pool.tile([D, QC], F32, tag="outp",
                                            name="out_ps")
                    acc = acc_pool.tile([128, GA, QC], F16, tag="acc",
                                        name="acc")
                    job = {"out_ps": out_ps, "acc": acc}
                    jobs[(h, qc)] = job
                v_sb = heads_sb[h][2]
                for i, kt in items:
                    nc.tensor.matmul(job["out_ps"][:], v_sb[:, kt, :],
                                     pt[:, i, :],
                                     start=(kt == 0), stop=(kt == NKT - 1))
                if fresh:
                    # first touch: copy instead of memset+add
                    nc.vector.tensor_copy(job["acc"][:, 0:n, :],
                                          pt[:, i0:i0 + n, :])
                    if n < GA:
                        nc.vector.memset(job["acc"][:, n:GA, :], 0.0)
                else:
                    nc.vector.tensor_add(job["acc"][:, 0:n, :],
                                         job["acc"][:, 0:n, :],
                                         pt[:, i0:i0 + n, :])
                if items[-1][1] == NKT - 1:
                    osb = osb_pool.tile([D, QC], F16, tag="osb")
                    nc.vector.tensor_copy(osb[:], job["out_ps"][:])
                    nc.sync.dma_start(out=out_d[h, qc], in_=osb[:])
                    # fold 4 den slots -> 2 to halve the DMA
                    nc.vector.tensor_add(job["acc"][:, 0:2, :],
                                         job["acc"][:, 0:2, :],
                                         job["acc"][:, 2:GA, :])
                    nc.sync.dma_start(out=acc_d[h, qc],
                                      in_=job["acc"][:, 0:2, :])
                    del jobs[(h, qc)]

        # PV/den deferred by TWO groups: keeps exp(g) -> scores(g+2) off
        # the PV path so ACT runs back-to-back.
        pending = []
        for gi, (use_a, group_chunks) in enumerate(groups):
            h0 = group_chunks[0][0]
            # prefetch head 1 two jobs ahead of first use
            if heads_sb[1] is None and group_chunks[0][1] >= NQC - 2 \
                    and h0 == 0:
                heads_sb[1] = load_head(1)
            n = len(group_chunks)
            pool = stA_pool if use_a else stB_pool
            st = pool.tile([128, GA if use_a else GB, QC], F32,
                           tag="stA" if use_a else "stB", name="st")
            for i, (h, qc, kt) in enumerate(group_chunks):
                qt_sb, kt_sb, _ = heads_sb[h]
                nc.tensor.matmul(
                    st[:, i, :],
                    kt_sb[:, kt * KT:(kt + 1) * KT],
                    qt_sb[:, qc * QC:(qc + 1) * QC],
                    start=True, stop=True)
            pt = pt_pool.tile([128, GA, QC], F16, tag="pt", name="pt")
            nc.scalar.activation(pt[:, 0:n, :], st[:, 0:n, :],
                                 mybir.ActivationFunctionType.Exp,
                                 scale=SCALE)
            pending.append((group_chunks, pt))
            if len(pending) > 2:
                emit_pv_den(*pending.pop(0))
        while pending:
            emit_pv_den(*pending.pop(0))

    nc.compile()
    return nc


def _install_ntff_hook():
    """Provide antenv.axon_hooks (absent in this image) so that
    run_bass_kernel_spmd(trace=True) can capture NTFF profiles via the
    axon .so."""
    try:
        from antenv.axon_hooks import get_axon_ntff_profile_hook  # noqa: F401
        return
    except ImportError:
        pass
    import contextlib
    import ctypes
    import types

    so_path = "/opt/axon/libaxon_pjrt.so"
    lib = ctypes.CDLL(so_path)
    if not hasattr(lib, "axon_start_nrt_profile"):
        return
    lib.axon_start_nrt_profile.argtypes = [
        ctypes.POINTER(ctypes.c_int64), ctypes.c_size_t]
    lib.axon_start_nrt_profile.restype = ctypes.c_int64
    lib.axon_stop_nrt_profile.argtypes = [ctypes.c_char_p]
    lib.axon_stop_nrt_profile.restype = ctypes.c_int64

    @contextlib.contextmanager
    def _hook(output_dir, device_ids):
        import jax
        jax.devices()
        if device_ids:
            ids = (ctypes.c_int64 * len(device_ids))(*device_ids)
            rc = lib.axon_start_nrt_profile(ids, len(device_ids))
        else:
            rc = lib.axon_start_nrt_profile(None, 0)
        if rc != 0:
            raise RuntimeError(f"axon_start_nrt_profile rc={rc}")
        try:
            yield
        finally:
            n = lib.axon_stop_nrt_profile(str(output_dir).encode())
            print(f"ntff profile: {n} file(s) written to {output_dir}")

    mod = types.ModuleType("antenv.axon_hooks")
    mod.get_axon_ntff_profile_hook = lambda: _hook
    mod.set_axon_ntff_profile_hook = lambda h: None
    import antenv
    sys.modules["antenv.axon_hooks"] = mod
    antenv.axon_hooks = mod


_CACHE = {}


def _get_program():
    key = "main"
    if key not in _CACHE:
        _CACHE[key] = build_program()
    return _CACHE[key]


def kernel(query, key, value, trace=False, **trace_kwargs):
    assert query.shape == (1, S, H, D)
    nc = _get_program()

    q = np.asarray(query, dtype=np.float32)[0]   # [S, H, D]
    k = np.asarray(key, dtype=np.float32)[0]
    v = np.asarray(value, dtype=np.float32)[0]

    in_maps = []
    for c in range(N_CORES):
        hs = slice(c * HPC, (c + 1) * HPC)
        # [S, h, D] -> [h, D, S] fp16
        qt = np.ascontiguousarray(
            q[:, hs, :].transpose(1, 2, 0)).astype(np.float16)
        kt = np.ascontiguousarray(
            k[:, hs, :].transpose(1, 2, 0)).astype(np.float16)
        # [S, h, D] -> [h, 128, NKT, D] fp16  (s = kt*128 + p)
        vv = np.ascontiguousarray(
            v[:, hs, :].transpose(1, 0, 2).reshape(HPC, NKT, 128, D)
            .transpose(0, 2, 1, 3)).astype(np.float16)
        in_maps.append({"qt": qt, "kt": kt, "v": vv})

    if trace:
        _install_ntff_hook()
    res = run_bass_kernel_spmd(nc, in_maps, core_ids=list(range(N_CORES)),
                               trace=trace, **trace_kwargs)

    out = np.empty((1, S, H, D), dtype=np.float32)
    for c in range(N_CORES):
        o = res.results[c]["out"].astype(np.float32)  # [HPC, NQC, D, QC]
        a = res.results[c]["acc"]    # [HPC, NQC, 128, 2, QC] f16
        den = a.astype(np.float32).sum(axis=(2, 3))  # [HPC, NQC, QC]
        on = o / den[:, :, None, :]                  # [HPC, NQC, D, QC]
        # -> [HPC, S, D]
        on = on.transpose(0, 1, 3, 2).reshape(HPC, S, D)
        for i in range(HPC):
            out[0, :, c * HPC + i, :] = on[i]
    if trace:
        kernel.last_results = res
    return out


# revision 16
# speedup vs baseline: 1.0725x; 1.0725x over previous
"""Trainium2 Bass kernel: full (non-causal) softmax attention.

Input:  query/key/value [1, 4096, 16, 128] f32 (B, S, H, D).
Output: [1, 4096, 16, 128] f32 = softmax(Q K^T / sqrt(D)) V per head.

Sharding: 16 heads over 8 cores -> 2 heads per core, no collectives.
Host pre-transposes Q,K per head to [D, S] in fp16; the device returns
the UN-normalized attention output [D, 512] per (head, q-chunk) plus
fp16 partial denominator accumulators [128, 4, 512]; the host reduces
the accumulators (sum over 128 k-lanes x 4 slots) and does the final
divide (cheap numpy).

Device pipeline (ACT-exp is the throughput floor, ~246us/core):
  - global stream of 512 score chunks (2 heads x 8 q-chunks x 32 kt),
    grouped into alternating 4-bank / 3-bank PSUM super-tiles
  - per group: PE writes scores (fp16 matmuls, N=512/bank), one big
    ACT exp (N=2048/1536, fp32 psum -> fp16 sbuf), then PE PV matmuls
    accumulate into a single out bank; DVE accumulates the softmax
    denominator with fp16 2x-mode adds into per-job accumulators.
  - software-pipelined by one group so PE never waits on ACT.
"""

import sys
from contextlib import ExitStack

import numpy as np

sys.path.insert(0, "/opt/trn_rl_repo")

import concourse.bacc as bacc
import concourse.bass as bass
import concourse.tile as tile
from concourse import mybir
from concourse.bass_utils import run_bass_kernel_spmd

N_CORES = 8
S = 4096
H = 16
D = 128
HPC = H // N_CORES   # heads per core = 2
QC = 512             # queries per job (one psum bank of fp32)
NQC = S // QC        # 8 q-chunks per head
KT = 128             # keys per score chunk (psum partition dim)
NKT = S // KT        # 32 key chunks
SCALE = float(D) ** -0.5
GA = 4               # chunks per group in buffer A (4 psum banks)
GB = 3               # chunks per group in buffer B (3 psum banks)

F32 = mybir.dt.float32
F16 = mybir.dt.float16


def _make_groups():
    """Global chunk stream split into alternating A/B groups.

    The remainder (size-1) group leads the stream: a 1-chunk group gets
    the first exp onto ACT as soon as its table load finishes.
    """
    chunks = [(h, qc, kt)
              for h in range(HPC) for qc in range(NQC) for kt in range(NKT)]
    # [1, 2, 2, 2, 2, 3, 4, 3, ...]: small leading groups start ACT
    # early and keep it fed while the cold PE ramps; the odd prefix sum
    # also keeps job boundaries off group boundaries (avoids a PE
    # hiccup on the shared out/acc resources every job).
    sizes = [1, 2, 2, 2, 2]
    use_a = False
    left = len(chunks) - sum(sizes)
    while left:
        n = min(GA if use_a else GB, left)
        sizes.append(n)
        left -= n
        use_a = not use_a
    groups = []
    i = 0
    for gi, n in enumerate(sizes):
        groups.append((gi % 2 == 0, chunks[i:i + n]))
        i += n
    return groups


def build_program():
    nc = bacc.Bacc("TRN2", target_bir_lowering=False, debug=False,
                   num_devices=N_CORES)

    qt_d = nc.dram_tensor("qt", [HPC, D, S], F16, kind="ExternalInput")
    kt_d = nc.dram_tensor("kt", [HPC, D, S], F16, kind="ExternalInput")
    v_d = nc.dram_tensor("v", [HPC, 128, NKT, D], F16, kind="ExternalInput")
    out_d = nc.dram_tensor("out", [HPC, NQC, D, QC], F16,
                           kind="ExternalOutput")
    acc_d = nc.dram_tensor("acc", [HPC, NQC, 128, 2, QC], F16,
                           kind="ExternalOutput")

    groups = _make_groups()

    with tile.TileContext(nc) as tc, ExitStack() as ctx:
        consts = ctx.enter_context(tc.tile_pool(name="consts", bufs=1))
        qkv_pool = ctx.enter_context(tc.tile_pool(name="qkv", bufs=2))
        pt_pool = ctx.enter_context(tc.tile_pool(name="pt", bufs=6))
        acc_pool = ctx.enter_context(tc.tile_pool(name="acc", bufs=3))
        osb_pool = ctx.enter_context(tc.tile_pool(name="osb", bufs=3))
        stA_pool = ctx.enter_context(
            tc.tile_pool(name="stA", bufs=1, space="PSUM"))
        stB_pool = ctx.enter_context(
            tc.tile_pool(name="stB", bufs=1, space="PSUM"))
        outp_pool = ctx.enter_context(
            tc.tile_pool(name="outp", bufs=1, space="PSUM"))

        # ---- warmup: ACT table load + PE HAM ramp while DMAs run ----
        wsrc = consts.tile([128, 16], F32, tag="wsrc")
        nc.vector.memset(wsrc[:], 0.0)
        wdst = consts.tile([128, 16], F16, tag="wdst")
        nc.scalar.activation(wdst[:], wsrc[:],
                             mybir.ActivationFunctionType.Exp, scale=SCALE)


        # ---- per-head staging ----
        def load_head(h, chunk_first=False):
            qt_sb = qkv_pool.tile([D, S], F16, tag="qt")
            kt_sb = qkv_pool.tile([D, S], F16, tag="kt")
            v_sb = qkv_pool.tile([128, NKT, D], F16, tag="v")
            if chunk_first:
                # staged so operands land in consumption order: kt/qt
                # prefix (first scores), v prefix (first PV), kt bulk
                # (first job walks all of kt), v bulk, qt remainder
                # (only needed at job 1, ~18us in)
                nc.sync.dma_start(out=kt_sb[:, 0:128], in_=kt_d[h][:, 0:128])
                nc.sync.dma_start(out=qt_sb[:, 0:QC], in_=qt_d[h][:, 0:QC])
                nc.sync.dma_start(out=kt_sb[:, 128:1024],
                                  in_=kt_d[h][:, 128:1024])
                nc.sync.dma_start(out=v_sb[:, 0:8, :], in_=v_d[h][:, 0:8, :])
                nc.sync.dma_start(out=kt_sb[:, 1024:], in_=kt_d[h][:, 1024:])
                nc.sync.dma_start(out=v_sb[:, 8:, :], in_=v_d[h][:, 8:, :])
                nc.sync.dma_start(out=qt_sb[:, QC:], in_=qt_d[h][:, QC:])
            else:
                nc.sync.dma_start(out=qt_sb[:], in_=qt_d[h])
                nc.sync.dma_start(out=kt_sb[:], in_=kt_d[h])
                nc.sync.dma_start(out=v_sb[:], in_=v_d[h])
            return qt_sb, kt_sb, v_sb

        heads_sb = [load_head(0, chunk_first=True), None]
        jobs = {}  # (h, qc) -> dict(out_ps, acc)

        def emit_pv_den(group_chunks, pt):
            # split the group by (h, qc) job; emit PV matmuls + den adds
            subs = []
            for i, (h, qc, kt) in enumerate(group_chunks):
                if subs and subs[-1][0] == (h, qc):
                    subs[-1][1].append((i, kt))
                else:
                    subs.append(((h, qc), [(i, kt)]))
            for (h, qc), items in subs:
                i0 = items[0][0]
                n = len(items)
                job = jobs.get((h, qc))
                fresh = job is None
                if fresh:
                    out_ps = outp_pool.tile([D, QC], F32, tag="outp",
                                            name="out_ps")
                    acc = acc_pool.tile([128, GA, QC], F16, tag="acc",
                                        name="acc")
                    job = {"out_ps": out_ps, "acc": acc}
                    jobs[(h, qc)] = job
                v_sb = heads_sb[h][2]
                for i, kt in items:
                    nc.tensor.matmul(job["out_ps"][:], v_sb[:, kt, :],
                                     pt[:, i, :],
                                     start=(kt == 0), stop=(kt == NKT - 1))
                if fresh:
                    # first touch: copy instead of memset+add
                    nc.vector.tensor_copy(job["acc"][:, 0:n, :],
                                          pt[:, i0:i0 + n, :])
                    if n < GA:
                        nc.vector.memset(job["acc"][:, n:GA, :], 0.0)
                else:
                    nc.vector.tensor_add(job["acc"][:, 0:n, :],
                                         job["acc"][:, 0:n, :],
                                         pt[:, i0:i0 + n, :])
                if items[-1][1] == NKT - 1:
                    osb = osb_pool.tile([D, QC], F16, tag="osb")
                    nc.vector.tensor_copy(osb[:], job["out_ps"][:])
                    nc.sync.dma_start(out=out_d[h, qc], in_=osb[:])
                    # fold 4 den slots -> 2 to halve the DMA
                    nc.vector.tensor_add(job["acc"][:, 0:2, :],
                                         job["acc"][:, 0:2, :],
                                         job["acc"][:, 2:GA, :])
                    nc.sync.dma_start(out=acc_d[h, qc],
                                      in_=job["acc"][:, 0:2, :])
                    del jobs[(h, qc)]

        # PV/den deferred by TWO groups: keeps exp(g) -> scores(g+2) off
        # the PV path so ACT runs back-to-back.
        pending = []
        for gi, (use_a, group_chunks) in enumerate(groups):
            h0 = group_chunks[0][0]
            # prefetch head 1 two jobs ahead of first use
            if heads_sb[1] is None and group_chunks[0][1] >= NQC - 2 \
                    and h0 == 0:
                heads_sb[1] = load_head(1)
            n = len(group_chunks)
            pool = stA_pool if use_a else stB_pool
            st = pool.tile([128, GA if use_a else GB, QC], F32,
                           tag="stA" if use_a else "stB", name="st")
            for i, (h, qc, kt) in enumerate(group_chunks):
                qt_sb, kt_sb, _ = heads_sb[h]
                nc.tensor.matmul(
                    st[:, i, :],
                    kt_sb[:, kt * KT:(kt + 1) * KT],
                    qt_sb[:, qc * QC:(qc + 1) * QC],
                    start=True, stop=True)
            pt = pt_pool.tile([128, GA, QC], F16, tag="pt", name="pt")
            nc.scalar.activation(pt[:, 0:n, :], st[:, 0:n, :],
                                 mybir.ActivationFunctionType.Exp,
                                 scale=SCALE)
            pending.append((group_chunks, pt))
            if len(pending) > 2:
                emit_pv_den(*pending.pop(0))
        while pending:
            emit_pv_den(*pending.pop(0))

    nc.compile()
    return nc


def _install_ntff_hook():
    """Provide antenv.axon_hooks (absent in this image) so that
    run_bass_kernel_spmd(trace=True) can capture NTFF profiles via the
    axon .so."""
    try:
        from antenv.axon_hooks import get_axon_ntff_profile_hook  # noqa: F401
        return
    except ImportError:
        pass
    import contextlib
    import ctypes
    import types

    so_path = "/opt/axon/libaxon_pjrt.so"
    lib = ctypes.CDLL(so_path)
    if not hasattr(lib, "axon_start_nrt_profile"):
        return
    lib.axon_start_nrt_profile.argtypes = [
        ctypes.POINTER(ctypes.c_int64), ctypes.c_size_t]
    lib.axon_start_nrt_profile.restype = ctypes.c_int64
    lib.axon_stop_nrt_profile.argtypes = [ctypes.c_char_p]
    lib.axon_stop_nrt_profile.restype = ctypes.c_int64

    @contextlib.contextmanager
    def _hook(output_dir, device_ids):
        import jax
        jax.devices()
        if device_ids:
            ids = (ctypes.c_int64 * len(device_ids))(*device_ids)
            rc = lib.axon_start_nrt_profile(ids, len(device_ids))
        else:
            rc = lib.axon_start_nrt_profile(None, 0)
        if rc != 0:
            raise RuntimeError(f"axon_start_nrt_profile rc={rc}")
        try:
            yield
        finally:
            n = lib.axon_stop_nrt_profile(str(output_dir).encode())
            print(f"ntff profile: {n} file(s) written to {output_dir}")

    mod = types.ModuleType("antenv.axon_hooks")
    mod.get_axon_ntff_profile_hook = lambda: _hook
    mod.set_axon_ntff_profile_hook = lambda h: None
    import antenv
    sys.modules["antenv.axon_hooks"] = mod
    antenv.axon_hooks = mod


_CACHE = {}


def _get_program():
    key = "main"
    if key not in _CACHE:
        _CACHE[key] = build_program()
    return _CACHE[key]


def kernel(query, key, value, trace=False, **trace_kwargs):
    assert query.shape == (1, S, H, D)
    nc = _get_program()

    q = np.asarray(query, dtype=np.float32)[0]   # [S, H, D]
    k = np.asarray(key, dtype=np.float32)[0]
    v = np.asarray(value, dtype=np.float32)[0]

    in_maps = []
    for c in range(N_CORES):
        hs = slice(c * HPC, (c + 1) * HPC)
        # [S, h, D] -> [h, D, S] fp16
        qt = np.ascontiguousarray(
            q[:, hs, :].transpose(1, 2, 0)).astype(np.float16)
        kt = np.ascontiguousarray(
            k[:, hs, :].transpose(1, 2, 0)).astype(np.float16)
        # [S, h, D] -> [h, 128, NKT, D] fp16  (s = kt*128 + p)
        vv = np.ascontiguousarray(
            v[:, hs, :].transpose(1, 0, 2).reshape(HPC, NKT, 128, D)
            .transpose(0, 2, 1, 3)).astype(np.float16)
        in_maps.append({"qt": qt, "kt": kt, "v": vv})

    if trace:
        _install_ntff_hook()
    res = run_bass_kernel_spmd(nc, in_maps, core_ids=list(range(N_CORES)),
                               trace=trace, **trace_kwargs)

    out = np.empty((1, S, H, D), dtype=np.float32)
    for c in range(N_CORES):
        o = res.results[c]["out"].astype(np.float32)  # [HPC, NQC, D, QC]
        a = res.results[c]["acc"]    # [HPC, NQC, 128, 2, QC] f16
        den = a.astype(np.float32).sum(axis=(2, 3))  # [HPC, NQC, QC]
        on = o / den[:, :, None, :]                  # [HPC, NQC, D, QC]
        # -> [HPC, S, D]
        on = on.transpose(0, 1, 3, 2).reshape(HPC, S, D)
        for i in range(HPC):
            out[0, :, c * HPC + i, :] = on[i]
    if trace:
        kernel.last_results = res
    return out


# revision 18
# speedup vs baseline: 1.0800x; 1.0070x over previous
"""Trainium2 Bass kernel: full (non-causal) softmax attention.

Input:  query/key/value [1, 4096, 16, 128] f32 (B, S, H, D).
Output: [1, 4096, 16, 128] f32 = softmax(Q K^T / sqrt(D)) V per head.

Sharding: 16 heads over 8 cores -> 2 heads per core, no collectives.
Host pre-transposes Q,K per head to [D, S] in fp16; the device returns
the UN-normalized attention output [D, 512] per (head, q-chunk) plus
fp16 partial denominator accumulators [128, 4, 512]; the host reduces
the accumulators (sum over 128 k-lanes x 4 slots) and does the final
divide (cheap numpy).

Device pipeline (ACT-exp is the throughput floor, ~246us/core):
  - global stream of 512 score chunks (2 heads x 8 q-chunks x 32 kt),
    grouped into alternating 4-bank / 3-bank PSUM super-tiles
  - per group: PE writes scores (fp16 matmuls, N=512/bank), one big
    ACT exp (N=2048/1536, fp32 psum -> fp16 sbuf), then PE PV matmuls
    accumulate into a single out bank; DVE accumulates the softmax
    denominator with fp16 2x-mode adds into per-job accumulators.
  - software-pipelined by one group so PE never waits on ACT.
"""

import sys
from contextlib import ExitStack

import numpy as np

sys.path.insert(0, "/opt/trn_rl_repo")

import concourse.bacc as bacc
import concourse.bass as bass
import concourse.tile as tile
from concourse import mybir
from concourse.bass_utils import run_bass_kernel_spmd

N_CORES = 8
S = 4096
H = 16
D = 128
HPC = H // N_CORES   # heads per core = 2
QC = 512             # queries per job (one psum bank of fp32)
NQC = S // QC        # 8 q-chunks per head
KT = 128             # keys per score chunk (psum partition dim)
NKT = S // KT        # 32 key chunks
SCALE = float(D) ** -0.5
GA = 4               # chunks per group in buffer A (4 psum banks)
GB = 3               # chunks per group in buffer B (3 psum banks)

F32 = mybir.dt.float32
F16 = mybir.dt.float16


def _make_groups():
    """Global chunk stream split into alternating A/B groups.

    The remainder (size-1) group leads the stream: a 1-chunk group gets
    the first exp onto ACT as soon as its table load finishes.
    """
    chunks = [(h, qc, kt)
              for h in range(HPC) for qc in range(NQC) for kt in range(NKT)]
    # [1, 2, 2, 2, 2, 3, 4, 3, ...]: small leading groups start ACT
    # early and keep it fed while the cold PE ramps; the odd prefix sum
    # also keeps job boundaries off group boundaries (avoids a PE
    # hiccup on the shared out/acc resources every job).
    sizes = [1, 2, 2, 2, 2]
    use_a = False
    left = len(chunks) - sum(sizes)
    while left:
        if left == 6:
            # short final groups shorten the post-last-exp tail chain
            sizes += [2, 2, 2]
            break
        n = min(GA if use_a else GB, left)
        sizes.append(n)
        left -= n
        use_a = not use_a
    groups = []
    i = 0
    for gi, n in enumerate(sizes):
        groups.append((gi % 2 == 0, chunks[i:i + n]))
        i += n
    return groups


def build_program():
    nc = bacc.Bacc("TRN2", target_bir_lowering=False, debug=False,
                   num_devices=N_CORES)

    qt_d = nc.dram_tensor("qt", [HPC, D, S], F16, kind="ExternalInput")
    kt_d = nc.dram_tensor("kt", [HPC, D, S], F16, kind="ExternalInput")
    v_d = nc.dram_tensor("v", [HPC, 128, NKT, D], F16, kind="ExternalInput")
    out_d = nc.dram_tensor("out", [HPC, NQC, D, QC], F16,
                           kind="ExternalOutput")
    acc_d = nc.dram_tensor("acc", [HPC, NQC, 128, 2, QC], F16,
                           kind="ExternalOutput")

    groups = _make_groups()

    with tile.TileContext(nc) as tc, ExitStack() as ctx:
        consts = ctx.enter_context(tc.tile_pool(name="consts", bufs=1))
        qkv_pool = ctx.enter_context(tc.tile_pool(name="qkv", bufs=2))
        pt_pool = ctx.enter_context(tc.tile_pool(name="pt", bufs=6))
        acc_pool = ctx.enter_context(tc.tile_pool(name="acc", bufs=3))
        osb_pool = ctx.enter_context(tc.tile_pool(name="osb", bufs=3))
        stA_pool = ctx.enter_context(
            tc.tile_pool(name="stA", bufs=1, space="PSUM"))
        stB_pool = ctx.enter_context(
            tc.tile_pool(name="stB", bufs=1, space="PSUM"))
        outp_pool = ctx.enter_context(
            tc.tile_pool(name="outp", bufs=1, space="PSUM"))

        # ---- warmup: ACT table load + PE HAM ramp while DMAs run ----
        wsrc = consts.tile([128, 16], F32, tag="wsrc")
        nc.vector.memset(wsrc[:], 0.0)
        wdst = consts.tile([128, 16], F16, tag="wdst")
        nc.scalar.activation(wdst[:], wsrc[:],
                             mybir.ActivationFunctionType.Exp, scale=SCALE)


        # ---- per-head staging ----
        def load_head(h, chunk_first=False):
            qt_sb = qkv_pool.tile([D, S], F16, tag="qt")
            kt_sb = qkv_pool.tile([D, S], F16, tag="kt")
            v_sb = qkv_pool.tile([128, NKT, D], F16, tag="v")
            if chunk_first:
                # staged so operands land in consumption order: kt/qt
                # prefix (first scores), v prefix (first PV), kt bulk
                # (first job walks all of kt), v bulk, qt remainder
                # (only needed at job 1, ~18us in). Few large calls:
                # each dma_start costs ~650ns of SP issue time.
                nc.sync.dma_start(out=kt_sb[:, 0:1536],
                                  in_=kt_d[h][:, 0:1536])
                nc.sync.dma_start(out=qt_sb[:, 0:QC], in_=qt_d[h][:, 0:QC])
                nc.sync.dma_start(out=v_sb[:, 0:8, :], in_=v_d[h][:, 0:8, :])
                nc.sync.dma_start(out=kt_sb[:, 1536:], in_=kt_d[h][:, 1536:])
                nc.sync.dma_start(out=v_sb[:, 8:, :], in_=v_d[h][:, 8:, :])
                nc.sync.dma_start(out=qt_sb[:, QC:], in_=qt_d[h][:, QC:])
            else:
                nc.sync.dma_start(out=qt_sb[:], in_=qt_d[h])
                nc.sync.dma_start(out=kt_sb[:], in_=kt_d[h])
                nc.sync.dma_start(out=v_sb[:], in_=v_d[h])
            return qt_sb, kt_sb, v_sb

        heads_sb = [load_head(0, chunk_first=True), None]
        jobs = {}  # (h, qc) -> dict(out_ps, acc)

        def emit_pv_den(group_chunks, pt):
            # split the group by (h, qc) job; emit PV matmuls + den adds
            subs = []
            for i, (h, qc, kt) in enumerate(group_chunks):
                if subs and subs[-1][0] == (h, qc):
                    subs[-1][1].append((i, kt))
                else:
                    subs.append(((h, qc), [(i, kt)]))
            for (h, qc), items in subs:
                i0 = items[0][0]
                n = len(items)
                job = jobs.get((h, qc))
                fresh = job is None
                if fresh:
                    out_ps = outp_pool.tile([D, QC], F32, tag="outp",
                                            name="out_ps")
                    acc = acc_pool.tile([128, GA, QC], F16, tag="acc",
                                        name="acc")
                    job = {"out_ps": out_ps, "acc": acc}
                    jobs[(h, qc)] = job
                v_sb = heads_sb[h][2]
                for i, kt in items:
                    nc.tensor.matmul(job["out_ps"][:], v_sb[:, kt, :],
                                     pt[:, i, :],
                                     start=(kt == 0), stop=(kt == NKT - 1))
                if fresh:
                    # first touch: copy instead of memset+add
                    nc.vector.tensor_copy(job["acc"][:, 0:n, :],
                                          pt[:, i0:i0 + n, :])
                    if n < GA:
                        nc.vector.memset(job["acc"][:, n:GA, :], 0.0)
                else:
                    nc.vector.tensor_add(job["acc"][:, 0:n, :],
                                         job["acc"][:, 0:n, :],
                                         pt[:, i0:i0 + n, :])
                if items[-1][1] == NKT - 1:
                    osb = osb_pool.tile([D, QC], F16, tag="osb")
                    nc.vector.tensor_copy(osb[:], job["out_ps"][:])
                    nc.sync.dma_start(out=out_d[h, qc], in_=osb[:])
                    # fold 4 den slots -> 2 to halve the DMA
                    nc.vector.tensor_add(job["acc"][:, 0:2, :],
                                         job["acc"][:, 0:2, :],
                                         job["acc"][:, 2:GA, :])
                    nc.sync.dma_start(out=acc_d[h, qc],
                                      in_=job["acc"][:, 0:2, :])
                    del jobs[(h, qc)]

        # PV/den deferred by TWO groups: keeps exp(g) -> scores(g+2) off
        # the PV path so ACT runs back-to-back.
        pending = []
        for gi, (use_a, group_chunks) in enumerate(groups):
            h0 = group_chunks[0][0]
            # prefetch head 1 two jobs ahead of first use
            if heads_sb[1] is None and group_chunks[0][1] >= NQC - 2 \
                    and h0 == 0:
                heads_sb[1] = load_head(1)
            n = len(group_chunks)
            pool = stA_pool if use_a else stB_pool
            st = pool.tile([128, GA if use_a else GB, QC], F32,
                           tag="stA" if use_a else "stB", name="st")
            for i, (h, qc, kt) in enumerate(group_chunks):
                qt_sb, kt_sb, _ = heads_sb[h]
                nc.tensor.matmul(
                    st[:, i, :],
                    kt_sb[:, kt * KT:(kt + 1) * KT],
                    qt_sb[:, qc * QC:(qc + 1) * QC],
                    start=True, stop=True)
            pt = pt_pool.tile([128, GA, QC], F16, tag="pt", name="pt")
            nc.scalar.activation(pt[:, 0:n, :], st[:, 0:n, :],
                                 mybir.ActivationFunctionType.Exp,
                                 scale=SCALE)
            pending.append((group_chunks, pt))
            if len(pending) > 2:
                emit_pv_den(*pending.pop(0))
        while pending:
            emit_pv_den(*pending.pop(0))

    nc.compile()
    return nc


def _install_ntff_hook():
    """Provide antenv.axon_hooks (absent in this image) so that
    run_bass_kernel_spmd(trace=True) can capture NTFF profiles via the
    axon .so."""
    try:
        from antenv.axon_hooks import get_axon_ntff_profile_hook  # noqa: F401
        return
    except ImportError:
        pass
    import contextlib
    import ctypes
    import types

    so_path = "/opt/axon/libaxon_pjrt.so"
    lib = ctypes.CDLL(so_path)
    if not hasattr(lib, "axon_start_nrt_profile"):
        return
    lib.axon_start_nrt_profile.argtypes = [
        ctypes.POINTER(ctypes.c_int64), ctypes.c_size_t]
    lib.axon_start_nrt_profile.restype = ctypes.c_int64
    lib.axon_stop_nrt_profile.argtypes = [ctypes.c_char_p]
    lib.axon_stop_nrt_profile.restype = ctypes.c_int64

    @contextlib.contextmanager
    def _hook(output_dir, device_ids):
        import jax
        jax.devices()
        if device_ids:
            ids = (ctypes.c_int64 * len(device_ids))(*device_ids)
            rc = lib.axon_start_nrt_profile(ids, len(device_ids))
        else:
            rc = lib.axon_start_nrt_profile(None, 0)
        if rc != 0:
            raise RuntimeError(f"axon_start_nrt_profile rc={rc}")
        try:
            yield
        finally:
            n = lib.axon_stop_nrt_profile(str(output_dir).encode())
            print(f"ntff profile: {n} file(s) written to {output_dir}")

    mod = types.ModuleType("antenv.axon_hooks")
    mod.get_axon_ntff_profile_hook = lambda: _hook
    mod.set_axon_ntff_profile_hook = lambda h: None
    import antenv
    sys.modules["antenv.axon_hooks"] = mod
    antenv.axon_hooks = mod


_CACHE = {}


def _get_program():
    key = "main"
    if key not in _CACHE:
        _CACHE[key] = build_program()
    return _CACHE[key]


def kernel(query, key, value, trace=False, **trace_kwargs):
    assert query.shape == (1, S, H, D)
    nc = _get_program()

    q = np.asarray(query, dtype=np.float32)[0]   # [S, H, D]
    k = np.asarray(key, dtype=np.float32)[0]
    v = np.asarray(value, dtype=np.float32)[0]

    in_maps = []
    for c in range(N_CORES):
        hs = slice(c * HPC, (c + 1) * HPC)
        # [S, h, D] -> [h, D, S] fp16
        qt = np.ascontiguousarray(
            q[:, hs, :].transpose(1, 2, 0)).astype(np.float16)
        kt = np.ascontiguousarray(
            k[:, hs, :].transpose(1, 2, 0)).astype(np.float16)
        # [S, h, D] -> [h, 128, NKT, D] fp16  (s = kt*128 + p)
        vv = np.ascontiguousarray(
            v[:, hs, :].transpose(1, 0, 2).reshape(HPC, NKT, 128, D)
            .transpose(0, 2, 1, 3)).astype(np.float16)
        in_maps.append({"qt": qt, "kt": kt, "v": vv})

    if trace:
        _install_ntff_hook()
    res = run_bass_kernel_spmd(nc, in_maps, core_ids=list(range(N_CORES)),
                               trace=trace, **trace_kwargs)

    out = np.empty((1, S, H, D), dtype=np.float32)
    for c in range(N_CORES):
        o = res.results[c]["out"].astype(np.float32)  # [HPC, NQC, D, QC]
        a = res.results[c]["acc"]    # [HPC, NQC, 128, 2, QC] f16
        den = a.astype(np.float32).sum(axis=(2, 3))  # [HPC, NQC, QC]
        on = o / den[:, :, None, :]                  # [HPC, NQC, D, QC]
        # -> [HPC, S, D]
        on = on.transpose(0, 1, 3, 2).reshape(HPC, S, D)
        for i in range(HPC):
            out[0, :, c * HPC + i, :] = on[i]
    if trace:
        kernel.last_results = res
    return out


# revision 20
# speedup vs baseline: 1.0803x; 1.0004x over previous
"""Trainium2 Bass kernel: full (non-causal) softmax attention.

Input:  query/key/value [1, 4096, 16, 128] f32 (B, S, H, D).
Output: [1, 4096, 16, 128] f32 = softmax(Q K^T / sqrt(D)) V per head.

Sharding: 16 heads over 8 cores -> 2 heads per core, no collectives.
Host pre-transposes Q,K per head to [D, S] in fp16; the device returns
the UN-normalized attention output [D, 512] per (head, q-chunk) plus
fp16 partial denominator accumulators [128, 4, 512]; the host reduces
the accumulators (sum over 128 k-lanes x 4 slots) and does the final
divide (cheap numpy).

Device pipeline (ACT-exp is the throughput floor, ~246us/core):
  - global stream of 512 score chunks (2 heads x 8 q-chunks x 32 kt),
    grouped into alternating 4-bank / 3-bank PSUM super-tiles
  - per group: PE writes scores (fp16 matmuls, N=512/bank), one big
    ACT exp (N=2048/1536, fp32 psum -> fp16 sbuf), then PE PV matmuls
    accumulate into a single out bank; DVE accumulates the softmax
    denominator with fp16 2x-mode adds into per-job accumulators.
  - software-pipelined by one group so PE never waits on ACT.
"""

import sys
from contextlib import ExitStack

import numpy as np

sys.path.insert(0, "/opt/trn_rl_repo")

import concourse.bacc as bacc
import concourse.bass as bass
import concourse.tile as tile
from concourse import mybir
from concourse.bass_utils import run_bass_kernel_spmd

N_CORES = 8
S = 4096
H = 16
D = 128
HPC = H // N_CORES   # heads per core = 2
QC = 512             # queries per job (one psum bank of fp32)
NQC = S // QC        # 8 q-chunks per head
KT = 128             # keys per score chunk (psum partition dim)
NKT = S // KT        # 32 key chunks
SCALE = float(D) ** -0.5
GA = 4               # chunks per group in buffer A (4 psum banks)
GB = 3               # chunks per group in buffer B (3 psum banks)

F32 = mybir.dt.float32
F16 = mybir.dt.float16


def _make_groups():
    """Global chunk stream split into alternating A/B groups.

    The remainder (size-1) group leads the stream: a 1-chunk group gets
    the first exp onto ACT as soon as its table load finishes.
    """
    chunks = [(h, qc, kt)
              for h in range(HPC) for qc in range(NQC) for kt in range(NKT)]
    # [1, 2, 2, 2, 2, 3, 4, 3, ...]: small leading groups start ACT
    # early and keep it fed while the cold PE ramps; the odd prefix sum
    # also keeps job boundaries off group boundaries (avoids a PE
    # hiccup on the shared out/acc resources every job).
    sizes = [1, 2, 2, 2, 2]
    use_a = False
    left = len(chunks) - sum(sizes)
    while left:
        if left == 6:
            # short final groups shorten the post-last-exp tail chain
            sizes += [2, 2, 2]
            break
        n = min(GA if use_a else GB, left)
        sizes.append(n)
        left -= n
        use_a = not use_a
    groups = []
    i = 0
    for gi, n in enumerate(sizes):
        groups.append((gi % 2 == 0, chunks[i:i + n]))
        i += n
    return groups


def build_program():
    nc = bacc.Bacc("TRN2", target_bir_lowering=False, debug=False,
                   num_devices=N_CORES)

    qt_d = nc.dram_tensor("qt", [HPC, D, S], F16, kind="ExternalInput")
    kt_d = nc.dram_tensor("kt", [HPC, D, S], F16, kind="ExternalInput")
    v_d = nc.dram_tensor("v", [HPC, 128, NKT, D], F16, kind="ExternalInput")
    out_d = nc.dram_tensor("out", [HPC, NQC, D, QC], F16,
                           kind="ExternalOutput")
    acc_d = nc.dram_tensor("acc", [HPC, NQC, 128, 2, QC], F16,
                           kind="ExternalOutput")

    groups = _make_groups()

    with tile.TileContext(nc) as tc, ExitStack() as ctx:
        consts = ctx.enter_context(tc.tile_pool(name="consts", bufs=1))
        qkv_pool = ctx.enter_context(tc.tile_pool(name="qkv", bufs=2))
        pt_pool = ctx.enter_context(tc.tile_pool(name="pt", bufs=6))
        acc_pool = ctx.enter_context(tc.tile_pool(name="acc", bufs=3))
        osb_pool = ctx.enter_context(tc.tile_pool(name="osb", bufs=3))
        stA_pool = ctx.enter_context(
            tc.tile_pool(name="stA", bufs=1, space="PSUM"))
        stB_pool = ctx.enter_context(
            tc.tile_pool(name="stB", bufs=1, space="PSUM"))
        outp_pool = ctx.enter_context(
            tc.tile_pool(name="outp", bufs=1, space="PSUM"))

        # ---- warmup: ACT table load + PE HAM ramp while DMAs run ----
        wsrc = consts.tile([128, 16], F32, tag="wsrc")
        nc.vector.memset(wsrc[:], 0.0)
        wdst = consts.tile([128, 16], F16, tag="wdst")
        nc.scalar.activation(wdst[:], wsrc[:],
                             mybir.ActivationFunctionType.Exp, scale=SCALE)


        # ---- per-head staging ----
        def load_head(h, chunk_first=False):
            qt_sb = qkv_pool.tile([D, S], F16, tag="qt")
            kt_sb = qkv_pool.tile([D, S], F16, tag="kt")
            v_sb = qkv_pool.tile([128, NKT, D], F16, tag="v")
            if chunk_first:
                # staged so operands land in consumption order, with the
                # three prefix loads issued from three different engine
                # queues in parallel (each dma_start costs ~650ns of
                # sequencer issue time; SP alone would serialize them)
                nc.sync.dma_start(out=kt_sb[:, 0:512], in_=kt_d[h][:, 0:512])
                nc.gpsimd.dma_start(out=qt_sb[:, 0:QC],
                                    in_=qt_d[h][:, 0:QC])
                nc.gpsimd.dma_start(out=v_sb[:, 0:8, :],
                                    in_=v_d[h][:, 0:8, :])
                nc.sync.dma_start(out=kt_sb[:, 512:1536],
                                  in_=kt_d[h][:, 512:1536])
                nc.sync.dma_start(out=kt_sb[:, 1536:], in_=kt_d[h][:, 1536:])
                nc.sync.dma_start(out=v_sb[:, 8:, :], in_=v_d[h][:, 8:, :])
                nc.sync.dma_start(out=qt_sb[:, QC:], in_=qt_d[h][:, QC:])
            else:
                nc.sync.dma_start(out=qt_sb[:], in_=qt_d[h])
                nc.sync.dma_start(out=kt_sb[:], in_=kt_d[h])
                nc.sync.dma_start(out=v_sb[:], in_=v_d[h])
            return qt_sb, kt_sb, v_sb

        heads_sb = [load_head(0, chunk_first=True), None]
        jobs = {}  # (h, qc) -> dict(out_ps, acc)

        def emit_pv_den(group_chunks, pt):
            # split the group by (h, qc) job; emit PV matmuls + den adds
            subs = []
            for i, (h, qc, kt) in enumerate(group_chunks):
                if subs and subs[-1][0] == (h, qc):
                    subs[-1][1].append((i, kt))
                else:
                    subs.append(((h, qc), [(i, kt)]))
            for (h, qc), items in subs:
                i0 = items[0][0]
                n = len(items)
                job = jobs.get((h, qc))
                fresh = job is None
                if fresh:
                    out_ps = outp_pool.tile([D, QC], F32, tag="outp",
                                            name="out_ps")
                    acc = acc_pool.tile([128, GA, QC], F16, tag="acc",
                                        name="acc")
                    job = {"out_ps": out_ps, "acc": acc}
                    jobs[(h, qc)] = job
                v_sb = heads_sb[h][2]
                for i, kt in items:
                    nc.tensor.matmul(job["out_ps"][:], v_sb[:, kt, :],
                                     pt[:, i, :],
                                     start=(kt == 0), stop=(kt == NKT - 1))
                if fresh:
                    # first touch: copy instead of memset+add
                    nc.vector.tensor_copy(job["acc"][:, 0:n, :],
                                          pt[:, i0:i0 + n, :])
                    if n < GA:
                        nc.vector.memset(job["acc"][:, n:GA, :], 0.0)
                else:
                    nc.vector.tensor_add(job["acc"][:, 0:n, :],
                                         job["acc"][:, 0:n, :],
                                         pt[:, i0:i0 + n, :])
                if items[-1][1] == NKT - 1:
                    osb = osb_pool.tile([D, QC], F16, tag="osb")
                    nc.vector.tensor_copy(osb[:], job["out_ps"][:])
                    nc.sync.dma_start(out=out_d[h, qc], in_=osb[:])
                    # fold 4 den slots -> 2 to halve the DMA
                    nc.vector.tensor_add(job["acc"][:, 0:2, :],
                                         job["acc"][:, 0:2, :],
                                         job["acc"][:, 2:GA, :])
                    nc.sync.dma_start(out=acc_d[h, qc],
                                      in_=job["acc"][:, 0:2, :])
                    del jobs[(h, qc)]

        # PV/den deferred by TWO groups: keeps exp(g) -> scores(g+2) off
        # the PV path so ACT runs back-to-back.
        pending = []
        for gi, (use_a, group_chunks) in enumerate(groups):
            h0 = group_chunks[0][0]
            # prefetch head 1 two jobs ahead of first use
            if heads_sb[1] is None and group_chunks[0][1] >= NQC - 2 \
                    and h0 == 0:
                heads_sb[1] = load_head(1)
            n = len(group_chunks)
            pool = stA_pool if use_a else stB_pool
            st = pool.tile([128, GA if use_a else GB, QC], F32,
                           tag="stA" if use_a else "stB", name="st")
            for i, (h, qc, kt) in enumerate(group_chunks):
                qt_sb, kt_sb, _ = heads_sb[h]
                nc.tensor.matmul(
                    st[:, i, :],
                    kt_sb[:, kt * KT:(kt + 1) * KT],
                    qt_sb[:, qc * QC:(qc + 1) * QC],
                    start=True, stop=True)
            pt = pt_pool.tile([128, GA, QC], F16, tag="pt", name="pt")
            nc.scalar.activation(pt[:, 0:n, :], st[:, 0:n, :],
                                 mybir.ActivationFunctionType.Exp,
                                 scale=SCALE)
            pending.append((group_chunks, pt))
            if len(pending) > 2:
                emit_pv_den(*pending.pop(0))
        while pending:
            emit_pv_den(*pending.pop(0))

    nc.compile()
    return nc


def _install_ntff_hook():
    """Provide antenv.axon_hooks (absent in this image) so that
    run_bass_kernel_spmd(trace=True) can capture NTFF profiles via the
    axon .so."""
    try:
        from antenv.axon_hooks import get_axon_ntff_profile_hook  # noqa: F401
        return
    except ImportError:
        pass
    import contextlib
    import ctypes
    import types

    so_path = "/opt/axon/libaxon_pjrt.so"
    lib = ctypes.CDLL(so_path)
    if not hasattr(lib, "axon_start_nrt_profile"):
        return
    lib.axon_start_nrt_profile.argtypes = [
        ctypes.POINTER(ctypes.c_int64), ctypes.c_size_t]
    lib.axon_start_nrt_profile.restype = ctypes.c_int64
    lib.axon_stop_nrt_profile.argtypes = [ctypes.c_char_p]
    lib.axon_stop_nrt_profile.restype = ctypes.c_int64

    @contextlib.contextmanager
    def _hook(output_dir, device_ids):
        import jax
        jax.devices()
        if device_ids:
            ids = (ctypes.c_int64 * len(device_ids))(*device_ids)
            rc = lib.axon_start_nrt_profile(ids, len(device_ids))
        else:
            rc = lib.axon_start_nrt_profile(None, 0)
        if rc != 0:
            raise RuntimeError(f"axon_start_nrt_profile rc={rc}")
        try:
            yield
        finally:
            n = lib.axon_stop_nrt_profile(str(output_dir).encode())
            print(f"ntff profile: {n} file(s) written to {output_dir}")

    mod = types.ModuleType("antenv.axon_hooks")
    mod.get_axon_ntff_profile_hook = lambda: _hook
    mod.set_axon_ntff_profile_hook = lambda h: None
    import antenv
    sys.modules["antenv.axon_hooks"] = mod
    antenv.axon_hooks = mod


_CACHE = {}


def _get_program():
    key = "main"
    if key not in _CACHE:
        _CACHE[key] = build_program()
    return _CACHE[key]


def kernel(query, key, value, trace=False, **trace_kwargs):
    assert query.shape == (1, S, H, D)
    nc = _get_program()

    q = np.asarray(query, dtype=np.float32)[0]   # [S, H, D]
    k = np.asarray(key, dtype=np.float32)[0]
    v = np.asarray(value, dtype=np.float32)[0]

    in_maps = []
    for c in range(N_CORES):
        hs = slice(c * HPC, (c + 1) * HPC)
        # [S, h, D] -> [h, D, S] fp16
        qt = np.ascontiguousarray(
            q[:, hs, :].transpose(1, 2, 0)).astype(np.float16)
        kt = np.ascontiguousarray(
            k[:, hs, :].transpose(1, 2, 0)).astype(np.float16)
        # [S, h, D] -> [h, 128, NKT, D] fp16  (s = kt*128 + p)
        vv = np.ascontiguousarray(
            v[:, hs, :].transpose(1, 0, 2).reshape(HPC, NKT, 128, D)
            .transpose(0, 2, 1, 3)).astype(np.float16)
        in_maps.append({"qt": qt, "kt": kt, "v": vv})

    if trace:
        _install_ntff_hook()
    res = run_bass_kernel_spmd(nc, in_maps, core_ids=list(range(N_CORES)),
                               trace=trace, **trace_kwargs)

    out = np.empty((1, S, H, D), dtype=np.float32)
    for c in range(N_CORES):
        o = res.results[c]["out"].astype(np.float32)  # [HPC, NQC, D, QC]
        a = res.results[c]["acc"]    # [HPC, NQC, 128, 2, QC] f16
        den = a.astype(np.float32).sum(axis=(2, 3))  # [HPC, NQC, QC]
        on = o / den[:, :, None, :]                  # [HPC, NQC, D, QC]
        # -> [HPC, S, D]
        on = on.transpose(0, 1, 3, 2).reshape(HPC, S, D)
        for i in range(HPC):
            out[0, :, c * HPC + i, :] = on[i]
    if trace:
        kernel.last_results = res
    return out
